# revision 8
# baseline (speedup 1.0000x reference)
"""Trainium2 Bass kernel for nn_Disentangler (gnn_message_passing).

Reference computation per timestamp t (T=16):
  xn   = LayerNorm_E(x[t])                          [16384, 128] -> use first 8192 rows
  tee  = segment_sum(xn[:8192] by node_idx[t])      [50000, 128]
  pool = blockmean_4(tee)                           [50000, 32]
  agg  = mean over basket slots of pool[stacked]    [64, 32]
  out  = LayerNorm_2048(agg.reshape(1, 2048))

Algebraic reformulation used here (all FP math on device):
  For token i with node n_i, let A[i, j] = (# occurrences of n_i among basket
  j's 782 slots) = BT[n_i, j] (a gather from the histogram BT of
  stacked_indices).  Then with per-token LN1 stats (m_i, r_i = rsqrt(var+eps)),
  q_i[c] = sum_{e in block c} x[i,e] * g1[e], sc[c] = sum_block g1,
  bb[c] = mean_block b1:

    agg[j, c] = (1/782) * [ sum_i A[i,j] * u_i[c]            (u = q * r/4)
                            - sc[c] * sum_i A[i,j] * w_i      (w = m * r/4)
                            + bb[c] * sum_i A[i,j] ]

  i.e. one token-contraction matmul  A^T @ [u | 1 | w]  per timestamp.

Sharding: data-parallel over T (2 timestamps per core, 8 cores).

Device pipeline per timestamp:
  1. dma_gather(transpose=True) of the needed x rows (bf16) -> xT [E=128, NT]
     (E-major directly, no on-chip transposes needed)
  2. sqT = xT*xT (DVE)
  3. per-128-token-chunk matmuls: lhsT=xT_chunk, rhs=[Wg|1|0]  -> per-token
     [q(32), sum_x, 0]; second matmul with lhsT=sqT_chunk -> sum_x2.
     Evacuate PSUM -> stats (token-major [128, CH, 34], bf16)
  4. tiny per-token ops -> r4 = rsqrt(var+eps)/4, u = q*r4, w = m*r4*... (DVE)
  5. dma_gather of BT rows by node id (split <32768 / >=32768 for int16)
     -> A [128, CH, 64] bf16
  6. 45 accumulating matmuls -> PSUM [64, 34] = [sum A*u | kappa | lambda]
  7. agg finalize + LayerNorm(2048) via two tiny matmuls for the global
     sums, output [64, 32] -> HBM.

Host does only index manipulation (join/histogram of integer index tensors,
int16 wrapping) and weight preprocessing (pool-fused ln1 gamma/beta, bf16
casts).
"""

import os
import sys

import ml_dtypes
import numpy as np

# ---------------------------------------------------------------- constants
T = 16
TOK = 16384
E = 128
N_NODE = 8192
NUM_NODES = 50000
COMP_LEN = 64  # J baskets
MAX_LEN = 782
COMP_DIM = 32  # C
EPS = 1e-5

N_CORES = 8
T_LOC = T // N_CORES  # 2 timestamps per core

# node-id relabel so that both int16 gather halves have a spare zero row
RELABEL_SRC = 32767
RELABEL_DST = 50001
ZLO = 32767           # zero row of BT for lo-half padding
ZHI_ABS = 50000       # zero row of BT for hi-half padding
ZHI = ZHI_ABS - 32768
BT_ROWS = 50304

# static token-list sizes (padded); ~5186 +- 44 kept tokens expected
S1 = 3712             # tokens whose node < 32768  (mean ~3400, sd ~41)
S2 = 2048             # tokens whose node >= 32768 (mean ~1786, sd ~37)
NT = S1 + S2          # 5760 = 45 * 128
CH = NT // 128        # 45 chunks
NSTAT = 34            # [q(32) | sum_x | sum_x2]
GRP = 15              # stats chunks per PSUM tile (15*34=510 fp32 <= bank)

_PROGRAM = None       # cached (nc,) tuple
LAST_RESULTS = None   # BassKernelResults of the last run (for test harness)

BF16 = ml_dtypes.bfloat16


def _build_program():
    import concourse.bacc as bacc
    import concourse.bass as bass
    import concourse.mybir as mybir
    import concourse.tile as tile

    f32 = mybir.dt.float32
    bf16 = mybir.dt.bfloat16
    i16 = mybir.dt.int16

    nc = bacc.Bacc("TRN2", target_bir_lowering=False, debug=False,
                   num_devices=N_CORES)

    xb_d = nc.dram_tensor("xb", [T_LOC * N_NODE, E], bf16, kind="ExternalInput")
    bt_d = nc.dram_tensor("bt", [BT_ROWS, E], bf16, kind="ExternalInput")
    xgidx_d = nc.dram_tensor("xg_idx", [T_LOC, 128, NT // 16], i16,
                             kind="ExternalInput")
    aloidx_d = nc.dram_tensor("alo_idx", [T_LOC, 128, S1 // 16], i16,
                              kind="ExternalInput")
    ahiidx_d = nc.dram_tensor("ahi_idx", [T_LOC, 128, S2 // 16], i16,
                              kind="ExternalInput")
    wstat_d = nc.dram_tensor("wstat", [E, NSTAT], bf16, kind="ExternalInput")
    sc_d = nc.dram_tensor("sc782", [COMP_LEN, COMP_DIM], f32,
                          kind="ExternalInput")
    bb_d = nc.dram_tensor("bb782", [COMP_LEN, COMP_DIM], f32,
                          kind="ExternalInput")
    g2_d = nc.dram_tensor("g2", [COMP_LEN, COMP_DIM], f32, kind="ExternalInput")
    b2_d = nc.dram_tensor("b2", [COMP_LEN, COMP_DIM], f32, kind="ExternalInput")
    out_d = nc.dram_tensor("out", [T_LOC, COMP_LEN, COMP_DIM], f32,
                           kind="ExternalOutput")

    with tile.TileContext(nc) as tc:
        with (
            tc.tile_pool(name="const", bufs=1) as cp,
            tc.tile_pool(name="main", bufs=2) as pool,
            tc.tile_pool(name="small", bufs=2) as sp,
            tc.tile_pool(name="ps", bufs=2, space=bass.MemorySpace.PSUM) as psp,
            tc.tile_pool(name="psc", bufs=2, space=bass.MemorySpace.PSUM) as pscp,
            tc.tile_pool(name="psde", bufs=1, space=bass.MemorySpace.PSUM) as psdep,
        ):
            # ---- constants
            wstat = cp.tile([E, NSTAT], bf16)
            nc.sync.dma_start(wstat[:], wstat_d.ap())
            sc = cp.tile([COMP_LEN, COMP_DIM], f32)
            nc.sync.dma_start(sc[:], sc_d.ap())
            bb = cp.tile([COMP_LEN, COMP_DIM], f32)
            nc.sync.dma_start(bb[:], bb_d.ap())
            g2 = cp.tile([COMP_LEN, COMP_DIM], f32)
            nc.sync.dma_start(g2[:], g2_d.ap())
            b2 = cp.tile([COMP_LEN, COMP_DIM], f32)
            nc.sync.dma_start(b2[:], b2_d.ap())
            ones64 = cp.tile([COMP_LEN, 1], f32)
            nc.gpsimd.memset(ones64[:], 1.0)
            sel2 = cp.tile([COMP_LEN, 2], f32)
            nc.gpsimd.memset(sel2[:], 0.0)
            nc.gpsimd.memset(sel2[0:COMP_DIM, 0:1], 1.0)
            nc.gpsimd.memset(sel2[COMP_DIM:COMP_LEN, 1:2], 1.0)
            epsb = cp.tile([128, 1], f32)
            nc.gpsimd.memset(epsb[:], EPS)

            xb_ap = xb_d.ap()
            bt_ap = bt_d.ap()
            bt_hi_ap = bt_d.ap()[32768:BT_ROWS, :]

            for t in range(T_LOC):
                # ---- index tiles
                xg_i = pool.tile([128, NT // 16], i16, tag="xgidx")
                nc.sync.dma_start(xg_i[:], xgidx_d.ap()[t])
                alo_i = pool.tile([128, S1 // 16], i16, tag="aloidx")
                nc.sync.dma_start(alo_i[:], aloidx_d.ap()[t])
                ahi_i = pool.tile([128, S2 // 16], i16, tag="ahiidx")
                nc.sync.dma_start(ahi_i[:], ahiidx_d.ap()[t])

                # ---- 1. gather x rows transposed -> xT [E, NT] bf16
                xT3 = pool.tile([128, 1, NT], bf16, tag="xT")
                nc.gpsimd.dma_gather(xT3[:], xb_ap, xg_i[:], NT, NT, E,
                                     transpose=True, single_packet=False)
                xT = xT3[:, 0, :]

                # ---- 5. gather BT rows -> A [128, CH, 128] bf16 (cols 0:64)
                a_sb = pool.tile([128, CH, E], bf16, tag="A")
                nc.gpsimd.dma_gather(a_sb[:, 0:S1 // 128, :], bt_ap,
                                     alo_i[:], S1, S1, E, single_packet=False)
                nc.gpsimd.dma_gather(a_sb[:, S1 // 128:CH, :], bt_hi_ap,
                                     ahi_i[:], S2, S2, E, single_packet=False)

                # ---- 2. squared x (for variance)
                sqT = pool.tile([128, NT], bf16, tag="sqT")
                nc.vector.tensor_mul(sqT[:], xT, xT)

                # ---- 3. per-token stats via PE
                stats = pool.tile([128, CH, NSTAT], bf16, tag="stats")
                for grp in range(CH // GRP):
                    ps = psp.tile([128, GRP, NSTAT], f32, tag="ps_stats")
                    for k in range(GRP):
                        g = grp * GRP + k
                        lo, hi = g * 128, (g + 1) * 128
                        nc.tensor.matmul(ps[:, k, :], xT[:, lo:hi], wstat[:],
                                         start=True, stop=True)
                        nc.tensor.matmul(ps[:, k, 33:34], sqT[:, lo:hi],
                                         wstat[:, 32:33], start=True, stop=True)
                    nc.vector.tensor_copy(
                        stats[:, grp * GRP:(grp + 1) * GRP, :], ps[:])

                # ---- 4. per-token scalars (all [128, CH], tiny)
                m_f = sp.tile([128, CH], f32, tag="m")
                nc.vector.tensor_scalar_mul(m_f[:], stats[:, :, 32], 1.0 / E)
                v_f = sp.tile([128, CH], f32, tag="v")
                nc.vector.tensor_scalar_mul(v_f[:], stats[:, :, 33], 1.0 / E)
                m2_f = sp.tile([128, CH], f32, tag="m2")
                nc.vector.tensor_mul(m2_f[:], m_f[:], m_f[:])
                nc.vector.tensor_sub(v_f[:], v_f[:], m2_f[:])
                sd_f = sp.tile([128, CH], f32, tag="sd")
                nc.scalar.activation(sd_f[:], v_f[:],
                                     mybir.ActivationFunctionType.Sqrt,
                                     bias=epsb[:])
                ri_f = sp.tile([128, CH], f32, tag="ri")
                nc.vector.reciprocal(ri_f[:], sd_f[:])
                r4_b = sp.tile([128, CH], bf16, tag="r4")
                nc.vector.tensor_scalar_mul(r4_b[:], ri_f[:], 0.25)
                w_f = sp.tile([128, CH], f32, tag="w")
                nc.vector.tensor_mul(w_f[:], m_f[:], ri_f[:])

                rhs2 = pool.tile([128, CH, NSTAT], bf16, tag="rhs2")
                nc.vector.tensor_mul(
                    rhs2[:, :, 0:COMP_DIM], stats[:, :, 0:COMP_DIM],
                    r4_b[:].unsqueeze(2).broadcast_to([128, CH, COMP_DIM]))
                nc.gpsimd.memset(rhs2[:, :, 32:33], 1.0)
                nc.vector.tensor_scalar_mul(rhs2[:, :, 33], w_f[:], 0.25)

                # ---- 6. token contraction
                psc = pscp.tile([COMP_LEN, NSTAT], f32, tag="psC")
                for g in range(CH):
                    nc.tensor.matmul(psc[:], a_sb[:, g, 0:COMP_LEN],
                                     rhs2[:, g, :],
                                     start=(g == 0), stop=(g == CH - 1))

                # ---- 7. agg finalize ([64, 32] fp32, tiny)
                cat = sp.tile([COMP_LEN, NSTAT], f32, tag="cat")
                nc.scalar.copy(cat[:], psc[:])
                t1 = sp.tile([COMP_LEN, COMP_DIM], f32, tag="t1")
                nc.vector.tensor_mul(
                    t1[:], cat[:, 33:34].broadcast_to([COMP_LEN, COMP_DIM]),
                    sc[:])
                t2 = sp.tile([COMP_LEN, COMP_DIM], f32, tag="t2")
                nc.vector.tensor_mul(
                    t2[:], cat[:, 32:33].broadcast_to([COMP_LEN, COMP_DIM]),
                    bb[:])
                nc.vector.tensor_sub(t2[:], t2[:], t1[:])
                t0 = sp.tile([COMP_LEN, COMP_DIM], f32, tag="t0")
                nc.vector.tensor_scalar_mul(t0[:], cat[:, 0:COMP_DIM],
                                            1.0 / MAX_LEN)
                cat2 = sp.tile([COMP_LEN, 2 * COMP_DIM], f32, tag="cat2")
                nc.vector.tensor_add(cat2[:, 0:COMP_DIM], t0[:], t2[:])
                nc.scalar.square(cat2[:, COMP_DIM:2 * COMP_DIM],
                                 cat2[:, 0:COMP_DIM])

                # ---- LN2 global sums via PE
                psd = psdep.tile([COMP_LEN, 1], f32, tag="psD")
                nc.tensor.matmul(psd[:], cat2[:], ones64[:],
                                 start=True, stop=True)
                sD = sp.tile([COMP_LEN, 1], f32, tag="sD")
                nc.vector.tensor_copy(sD[:], psd[:])
                pse = psdep.tile([1, 2], f32, tag="psE")
                nc.tensor.matmul(pse[:], sD[:], sel2[:], start=True, stop=True)
                sE = sp.tile([1, 2], f32, tag="sE")
                nc.vector.tensor_copy(sE[:], pse[:])
                bS = sp.tile([COMP_LEN, 2], f32, tag="bS")
                nc.gpsimd.partition_broadcast(bS[:], sE[:], channels=COMP_LEN)

                NTOT = float(COMP_LEN * COMP_DIM)
                mu = sp.tile([COMP_LEN, 1], f32, tag="mu")
                nc.vector.tensor_scalar_mul(mu[:], bS[:, 0:1], 1.0 / NTOT)
                ex2 = sp.tile([COMP_LEN, 1], f32, tag="ex2")
                nc.vector.tensor_scalar_mul(ex2[:], bS[:, 1:2], 1.0 / NTOT)
                mu2 = sp.tile([COMP_LEN, 1], f32, tag="mu2")
                nc.vector.tensor_mul(mu2[:], mu[:], mu[:])
                nc.vector.tensor_sub(ex2[:], ex2[:], mu2[:])
                sd2 = sp.tile([COMP_LEN, 1], f32, tag="sd2")
                nc.scalar.activation(sd2[:], ex2[:],
                                     mybir.ActivationFunctionType.Sqrt,
                                     bias=epsb[0:COMP_LEN, :])
                rr = sp.tile([COMP_LEN, 1], f32, tag="rr")
                nc.vector.reciprocal(rr[:], sd2[:])

                obuf = sp.tile([COMP_LEN, COMP_DIM], f32, tag="obuf")
                nc.vector.tensor_scalar(obuf[:], cat2[:, 0:COMP_DIM],
                                        mu[:], rr[:],
                                        mybir.AluOpType.subtract,
                                        mybir.AluOpType.mult)
                nc.vector.tensor_mul(obuf[:], obuf[:], g2[:])
                nc.vector.tensor_add(obuf[:], obuf[:], b2[:])

                nc.sync.dma_start(out_d.ap()[t], obuf[:])

    nc.compile()
    return nc


def _get_program():
    global _PROGRAM
    if _PROGRAM is None:
        _PROGRAM = _build_program()
    return _PROGRAM


def _wrap_idx(arr):
    """[n] int array -> [128, n/16] int16 tile data (i = s*16 + p, replicated
    across the 8 GPSIMD cores' 16-partition blocks)."""
    n = arr.shape[0]
    assert n % 16 == 0
    w = arr.reshape(n // 16, 16).T.astype(np.int16)  # [16, n/16]
    return np.tile(w, (8, 1))


def _prepare_inputs(x, ln1_g, ln1_b, ln2_g, ln2_b, node_idx, stacked_indices):
    """Host-side index preprocessing + weight prep. Returns list of in_maps."""
    node_idx = np.asarray(node_idx).astype(np.int64)
    stacked = np.asarray(stacked_indices).astype(np.int64)
    x = np.asarray(x, dtype=np.float32)
    ln1_g = np.asarray(ln1_g, dtype=np.float32)
    ln1_b = np.asarray(ln1_b, dtype=np.float32)
    ln2_g = np.asarray(ln2_g, dtype=np.float32)
    ln2_b = np.asarray(ln2_b, dtype=np.float32)

    # relabel node id RELABEL_SRC -> RELABEL_DST (consistently on both sides)
    node_re = np.where(node_idx == RELABEL_SRC, RELABEL_DST, node_idx)
    stacked_re = np.where(stacked == RELABEL_SRC, RELABEL_DST, stacked)

    # histogram BT[n, j] = count of node n in basket j
    bt = np.zeros((BT_ROWS, E), dtype=np.float32)
    j_ids = np.broadcast_to(np.arange(COMP_LEN)[:, None], stacked_re.shape)
    np.add.at(bt, (stacked_re.ravel(), j_ids.ravel()), 1.0)
    bt[ZLO, :] = 0.0
    bt[ZHI_ABS, :] = 0.0
    bt_bf = bt.astype(BF16)

    in_basket = np.zeros(BT_ROWS, dtype=bool)
    in_basket[np.unique(stacked_re)] = True

    # weight prep
    wstat = np.zeros((E, NSTAT), dtype=np.float32)
    cols = np.arange(E) // 4
    wstat[np.arange(E), cols] = ln1_g
    wstat[:, 32] = 1.0
    wstat_bf = wstat.astype(BF16)
    sc = ln1_g.reshape(COMP_DIM, 4).sum(1)          # sum_block g
    bbv = ln1_b.reshape(COMP_DIM, 4).mean(1)        # mean_block b
    sc782 = np.broadcast_to(sc / MAX_LEN, (COMP_LEN, COMP_DIM)).copy()
    bb782 = np.broadcast_to(bbv / MAX_LEN, (COMP_LEN, COMP_DIM)).copy()
    g2 = ln2_g.reshape(COMP_LEN, COMP_DIM).copy()
    b2 = ln2_b.reshape(COMP_LEN, COMP_DIM).copy()

    in_maps = []
    for core in range(N_CORES):
        ts = range(core * T_LOC, (core + 1) * T_LOC)
        xg_l, alo_l, ahi_l = [], [], []
        for ti, tg in enumerate(ts):
            nt = node_re[tg, :N_NODE]
            kept = np.flatnonzero(in_basket[nt])
            nk = nt[kept]
            lo_sel = nk < 32768
            lo_tok, lo_node = kept[lo_sel], nk[lo_sel]
            hi_tok, hi_node = kept[~lo_sel], nk[~lo_sel] - 32768
            n1, n2 = len(lo_tok), len(hi_tok)
            if n1 > S1 or n2 > S2:
                print(f"WARNING: token overflow n1={n1} n2={n2}",
                      file=sys.stderr)
                lo_tok, lo_node = lo_tok[:S1], lo_node[:S1]
                hi_tok, hi_node = hi_tok[:S2], hi_node[:S2]
                n1, n2 = len(lo_tok), len(hi_tok)
            xg = np.full(NT, ti * N_NODE, dtype=np.int64)
            xg[0:n1] = lo_tok + ti * N_NODE
            xg[S1:S1 + n2] = hi_tok + ti * N_NODE
            alo = np.full(S1, ZLO, dtype=np.int64)
            alo[0:n1] = lo_node
            ahi = np.full(S2, ZHI, dtype=np.int64)
            ahi[0:n2] = hi_node
            xg_l.append(_wrap_idx(xg))
            alo_l.append(_wrap_idx(alo))
            ahi_l.append(_wrap_idx(ahi))
        xb = x[list(ts), :N_NODE, :].reshape(T_LOC * N_NODE, E).astype(BF16)
        in_maps.append({
            "xb": xb,
            "bt": bt_bf,
            "xg_idx": np.stack(xg_l),
            "alo_idx": np.stack(alo_l),
            "ahi_idx": np.stack(ahi_l),
            "wstat": wstat_bf,
            "sc782": sc782.astype(np.float32),
            "bb782": bb782.astype(np.float32),
            "g2": g2, "b2": b2,
        })
    return in_maps


def kernel(x, ln1_g, ln1_b, ln2_g, ln2_b, node_idx, stacked_indices,
           n_node=N_NODE, num_nodes=NUM_NODES):
    global LAST_RESULTS
    from concourse.bass_utils import run_bass_kernel_spmd

    nc = _get_program()
    in_maps = _prepare_inputs(x, ln1_g, ln1_b, ln2_g, ln2_b, node_idx,
                              stacked_indices)

    if os.environ.get("KERNEL_SIM"):
        outs = _run_sim(nc, in_maps)
    else:
        res = run_bass_kernel_spmd(
            nc, in_maps, core_ids=list(range(N_CORES)),
            trace=bool(os.environ.get("KERNEL_TRACE")),
        )
        LAST_RESULTS = res
        outs = [r["out"] for r in res.results]

    full = np.concatenate(outs, axis=0)           # [16, 64, 32]
    return full.reshape(T, 1, COMP_LEN * COMP_DIM).astype(np.float32)


def _run_sim(nc, in_maps):
    """CoreSim path (KERNEL_SIM=1): simulate each core serially."""
    from concourse.bass_interp import CoreSim
    outs = []
    ncores = int(os.environ.get("KERNEL_SIM_CORES", "1"))
    for core, im in enumerate(in_maps[:ncores]):
        sim = CoreSim(nc, trace=False)
        for k, v in im.items():
            sim.tensor(k)[:] = v
        sim.simulate(check_with_hw=False)
        outs.append(np.array(sim.tensor("out")))
    # remaining cores: numpy emulation of the same math for shape-completeness
    for core in range(ncores, len(in_maps)):
        outs.append(np.zeros((T_LOC, COMP_LEN, COMP_DIM), np.float32))
    return outs


# revision 14
# speedup vs baseline: 2.5824x; 2.5824x over previous
"""Trainium2 Bass kernel for nn_Disentangler (gnn_message_passing).

Reference computation per timestamp t (T=16):
  xn   = LayerNorm_E(x[t])                 [16384, 128] -> first 8192 rows used
  tee  = segment_sum(xn[:8192] by node_idx[t])      [50000, 128]
  pool = blockmean_4(tee)                           [50000, 32]
  agg  = mean over basket slots of pool[stacked]    [64, 32]
  out  = LayerNorm_2048(agg.reshape(1, 2048))

Algebraic reformulation (all FP math on x happens on device):
  For token i with node n_i, A[i, j] = (# occurrences of n_i among basket j's
  782 slots) — an integer count matrix derived purely from the two index
  tensors (host-side index preprocessing).  With per-token LN1 stats
  (m_i, r_i = rsqrt(var_i+eps)), q_i[c] = sum_{e in block c} x[i,e]*g1[e],
  sc[c] = sum_block g1, bb[c] = mean_block b1:

    agg[j, c] = (1/782) * [ sum_i A[i,j]*u_i[c]        (u = q * r/4)
                            - sc[c] * sum_i A[i,j]*w_i  (w = m * r/4)
                            + bb[c] * sum_i A[i,j] ]

  i.e. one token-contraction matmul  A^T @ [u | 1 | w]  per timestamp.

Sharding: data-parallel over T (2 timestamps per core, 8 cores).

Device pipeline per timestamp (no GPSIMD gathers — SWDGE desc-gen for
large dma_gathers measured ~8-16 ns/idx on the Q7 and dominated v1):
  1. xT [E=128, 8192] bf16 <- HWDGE dma_start_transpose of x[t] (xbar)
  2. sqT = xT*xT (DVE, one pass)
  3. stats: 16 matmuls psA[34,512] = wstat^T @ xT-chunk (wstat = [Wg|1|0]
     stationary) -> per-token [q(32), sum_x]; 16 matmuls with ones lhsT on
     sqT write sum_x2 into one partition-packed PSUM bank [16, 512].
  4. back-transpose stats to token-major: 64 PE transposes [34,128]->[128,34];
     4 PE transposes [16,128]->[128,16] reassemble sum_x2.
  5. tiny token-major DVE/ACT ops -> r4, u, w  (rhs2 = [u | 1 | w] bf16)
  6. 64 accumulating matmuls psC[64,34] = A-chunk^T @ rhs2-chunk
  7. agg finalize + LayerNorm(2048) (global sums via two tiny matmuls),
     output [64, 32] f32 -> HBM.
"""

import os
import sys

import ml_dtypes
import numpy as np

# ---------------------------------------------------------------- constants
T = 16
TOK = 16384
E = 128
N_NODE = 8192
NUM_NODES = 50000
COMP_LEN = 64   # J baskets
MAX_LEN = 782
COMP_DIM = 32   # C
EPS = 1e-5

N_CORES = 8
T_LOC = T // N_CORES   # 2 timestamps per core

NT = N_NODE            # tokens per timestamp
CH = NT // 128         # 64 token chunks
NK = NT // 512         # 16 stats matmul chunks
NSTAT = 34             # wstat cols: [q(32) | ones | zero]
BTR = 8                # back-transposes batched per PSUM tile

_PROGRAM = None
LAST_RESULTS = None    # BassKernelResults of the last run (for test harness)

BF16 = ml_dtypes.bfloat16


def _build_program():
    import concourse.bacc as bacc
    import concourse.bass as bass
    import concourse.mybir as mybir
    import concourse.tile as tile
    from concourse import masks

    f32 = mybir.dt.float32
    bf16 = mybir.dt.bfloat16

    nc = bacc.Bacc("TRN2", target_bir_lowering=False, debug=False,
                   num_devices=N_CORES)

    xb_d = nc.dram_tensor("xb", [T_LOC, NT, E], bf16, kind="ExternalInput")
    am_d = nc.dram_tensor("am", [T_LOC, 128, CH, COMP_LEN], bf16,
                          kind="ExternalInput")
    wstat_d = nc.dram_tensor("wstat", [E, NSTAT], bf16, kind="ExternalInput")
    sc_d = nc.dram_tensor("sc782", [COMP_LEN, COMP_DIM], f32,
                          kind="ExternalInput")
    bb_d = nc.dram_tensor("bb782", [COMP_LEN, COMP_DIM], f32,
                          kind="ExternalInput")
    g2_d = nc.dram_tensor("g2", [COMP_LEN, COMP_DIM], f32, kind="ExternalInput")
    b2_d = nc.dram_tensor("b2", [COMP_LEN, COMP_DIM], f32, kind="ExternalInput")
    out_d = nc.dram_tensor("out", [T_LOC, COMP_LEN, COMP_DIM], f32,
                           kind="ExternalOutput")

    with tile.TileContext(nc) as tc:
        with (
            tc.tile_pool(name="const", bufs=1) as cp,
            tc.tile_pool(name="main", bufs=2) as pool,
            tc.tile_pool(name="small", bufs=2) as sp,
            tc.tile_pool(name="ps", bufs=3, space=bass.MemorySpace.PSUM) as psp,
            tc.tile_pool(name="pst", bufs=2, space=bass.MemorySpace.PSUM) as pstp,
            tc.tile_pool(name="psc", bufs=2, space=bass.MemorySpace.PSUM) as pscp,
            tc.tile_pool(name="psde", bufs=1, space=bass.MemorySpace.PSUM) as psdep,
        ):
            # ---- constants
            wstat = cp.tile([E, NSTAT], bf16)
            nc.sync.dma_start(wstat[:], wstat_d.ap())
            sc = cp.tile([COMP_LEN, COMP_DIM], f32)
            nc.sync.dma_start(sc[:], sc_d.ap())
            bb = cp.tile([COMP_LEN, COMP_DIM], f32)
            nc.sync.dma_start(bb[:], bb_d.ap())
            g2 = cp.tile([COMP_LEN, COMP_DIM], f32)
            nc.sync.dma_start(g2[:], g2_d.ap())
            b2 = cp.tile([COMP_LEN, COMP_DIM], f32)
            nc.sync.dma_start(b2[:], b2_d.ap())
            ones64 = cp.tile([COMP_LEN, 1], f32)
            nc.gpsimd.memset(ones64[:], 1.0)
            sel2 = cp.tile([COMP_LEN, 2], f32)
            nc.gpsimd.memset(sel2[:], 0.0)
            nc.gpsimd.memset(sel2[0:COMP_DIM, 0:1], 1.0)
            nc.gpsimd.memset(sel2[COMP_DIM:COMP_LEN, 1:2], 1.0)
            epsb = cp.tile([128, 1], f32)
            nc.gpsimd.memset(epsb[:], EPS)
            ident = cp.tile([128, 128], bf16)
            masks.make_identity(nc, ident[:])
            # [zero | ones] selector: lhsT for the sum_x2 row (see below)
            ssqsel = cp.tile([E, 2], bf16)
            nc.gpsimd.memset(ssqsel[:, 0:1], 0.0)
            nc.gpsimd.memset(ssqsel[:, 1:2], 1.0)

            for t in range(T_LOC):
                # ---- 1. transposed load of x[t] (HWDGE xbar)
                xT = pool.tile([128, NT], bf16, tag="xT")
                nc.sync.dma_start_transpose(xT[:], xb_d.ap()[t])

                # ---- A matrix (host-prepared counts, chunk layout)
                a_sb = pool.tile([128, CH, COMP_LEN], bf16, tag="A")
                nc.sync.dma_start(a_sb[:], am_d.ap()[t])

                # ---- 2. squared x
                sqT = pool.tile([128, NT], bf16, tag="sqT")
                nc.vector.tensor_mul(sqT[:], xT[:], xT[:])

                # ---- 3. stats matmuls (N=512): rows 0-31 q, 32 sum_x,
                # 33 sum_x2.  Per chunk: first [0|ssq] lands on rows 32-33
                # (base partition 32), then the 33-col wstat matmul at base 0
                # overwrites row 32 with sum_x (program order => WAW safe).
                stats_e = pool.tile([NSTAT, NT], bf16, tag="stats_e")
                for k in range(NK):
                    ps = psp.tile([NSTAT, 512], f32, tag="psA")
                    nc.tensor.matmul(ps[32:34, :], ssqsel[:],
                                     sqT[:, k * 512:(k + 1) * 512],
                                     start=True, stop=True)
                    nc.tensor.matmul(ps[0:33, :], wstat[:, 0:33],
                                     xT[:, k * 512:(k + 1) * 512],
                                     start=True, stop=True)
                    ev = stats_e[:, k * 512:(k + 1) * 512]
                    if k % 2 == 0:
                        nc.scalar.copy(ev, ps[:])
                    else:
                        nc.vector.tensor_copy(ev, ps[:])

                # ---- 4. back-transpose stats to token-major
                stats_tok = pool.tile([128, CH, NSTAT], bf16, tag="stats_tok")
                for g0 in range(0, CH, BTR):
                    pstk = pstp.tile([128, BTR, NSTAT], bf16, tag="psTk")
                    for j in range(BTR):
                        g = g0 + j
                        nc.tensor.transpose(
                            pstk[:, j, :],
                            stats_e[:, g * 128:(g + 1) * 128],
                            ident[0:NSTAT, 0:NSTAT])
                    nc.vector.tensor_copy(stats_tok[:, g0:g0 + BTR, :], pstk[:])

                # ---- 5. per-token scalars (all [128, CH], tiny)
                m_f = sp.tile([128, CH], f32, tag="m")
                nc.vector.tensor_scalar_mul(m_f[:], stats_tok[:, :, 32], 1.0 / E)
                v_f = sp.tile([128, CH], f32, tag="v")
                nc.vector.tensor_scalar_mul(v_f[:], stats_tok[:, :, 33], 1.0 / E)
                m2_f = sp.tile([128, CH], f32, tag="m2")
                nc.vector.tensor_mul(m2_f[:], m_f[:], m_f[:])
                nc.vector.tensor_sub(v_f[:], v_f[:], m2_f[:])
                sd_f = sp.tile([128, CH], f32, tag="sd")
                nc.scalar.activation(sd_f[:], v_f[:],
                                     mybir.ActivationFunctionType.Sqrt,
                                     bias=epsb[:])
                ri_f = sp.tile([128, CH], f32, tag="ri")
                nc.vector.reciprocal(ri_f[:], sd_f[:])
                r4_b = sp.tile([128, CH], bf16, tag="r4")
                nc.vector.tensor_scalar_mul(r4_b[:], ri_f[:], 0.25)
                w_f = sp.tile([128, CH], f32, tag="w")
                nc.vector.tensor_mul(w_f[:], m_f[:], ri_f[:])

                rhs2 = pool.tile([128, CH, NSTAT], bf16, tag="rhs2")
                nc.vector.tensor_mul(
                    rhs2[:, :, 0:COMP_DIM], stats_tok[:, :, 0:COMP_DIM],
                    r4_b[:].unsqueeze(2).broadcast_to([128, CH, COMP_DIM]))
                nc.gpsimd.memset(rhs2[:, :, 32:33], 1.0)
                nc.vector.tensor_scalar_mul(rhs2[:, :, 33], w_f[:], 0.25)

                # ---- 6. token contraction
                psc = pscp.tile([COMP_LEN, NSTAT], f32, tag="psC")
                for g in range(CH):
                    nc.tensor.matmul(psc[:], a_sb[:, g, :], rhs2[:, g, :],
                                     start=(g == 0), stop=(g == CH - 1))

                # ---- 7. agg finalize ([64, 32] fp32, tiny)
                cat = sp.tile([COMP_LEN, NSTAT], f32, tag="cat")
                nc.scalar.copy(cat[:], psc[:])
                t1 = sp.tile([COMP_LEN, COMP_DIM], f32, tag="t1")
                nc.vector.tensor_mul(
                    t1[:], cat[:, 33:34].broadcast_to([COMP_LEN, COMP_DIM]),
                    sc[:])
                t2 = sp.tile([COMP_LEN, COMP_DIM], f32, tag="t2")
                nc.vector.tensor_mul(
                    t2[:], cat[:, 32:33].broadcast_to([COMP_LEN, COMP_DIM]),
                    bb[:])
                nc.vector.tensor_sub(t2[:], t2[:], t1[:])
                t0 = sp.tile([COMP_LEN, COMP_DIM], f32, tag="t0")
                nc.vector.tensor_scalar_mul(t0[:], cat[:, 0:COMP_DIM],
                                            1.0 / MAX_LEN)
                cat2 = sp.tile([COMP_LEN, 2 * COMP_DIM], f32, tag="cat2")
                nc.vector.tensor_add(cat2[:, 0:COMP_DIM], t0[:], t2[:])
                nc.scalar.square(cat2[:, COMP_DIM:2 * COMP_DIM],
                                 cat2[:, 0:COMP_DIM])

                # ---- LN2 global sums via PE
                psd = psdep.tile([COMP_LEN, 1], f32, tag="psDE")
                nc.tensor.matmul(psd[:], cat2[:], ones64[:],
                                 start=True, stop=True)
                sD = sp.tile([COMP_LEN, 1], f32, tag="sD")
                nc.vector.tensor_copy(sD[:], psd[:])
                pse = psdep.tile([1, 2], f32, tag="psDE")
                nc.tensor.matmul(pse[:], sD[:], sel2[:], start=True, stop=True)
                sE = sp.tile([1, 2], f32, tag="sE")
                nc.vector.tensor_copy(sE[:], pse[:])
                bS = sp.tile([COMP_LEN, 2], f32, tag="bS")
                nc.gpsimd.partition_broadcast(bS[:], sE[:], channels=COMP_LEN)

                NTOT = float(COMP_LEN * COMP_DIM)
                mu = sp.tile([COMP_LEN, 1], f32, tag="mu")
                nc.vector.tensor_scalar_mul(mu[:], bS[:, 0:1], 1.0 / NTOT)
                ex2 = sp.tile([COMP_LEN, 1], f32, tag="ex2")
                nc.vector.tensor_scalar_mul(ex2[:], bS[:, 1:2], 1.0 / NTOT)
                mu2 = sp.tile([COMP_LEN, 1], f32, tag="mu2")
                nc.vector.tensor_mul(mu2[:], mu[:], mu[:])
                nc.vector.tensor_sub(ex2[:], ex2[:], mu2[:])
                sd2 = sp.tile([COMP_LEN, 1], f32, tag="sd2")
                nc.scalar.activation(sd2[:], ex2[:],
                                     mybir.ActivationFunctionType.Sqrt,
                                     bias=epsb[0:COMP_LEN, :])
                rr = sp.tile([COMP_LEN, 1], f32, tag="rr")
                nc.vector.reciprocal(rr[:], sd2[:])

                obuf = sp.tile([COMP_LEN, COMP_DIM], f32, tag="obuf")
                nc.vector.tensor_scalar(obuf[:], cat2[:, 0:COMP_DIM],
                                        mu[:], rr[:],
                                        mybir.AluOpType.subtract,
                                        mybir.AluOpType.mult)
                nc.vector.tensor_mul(obuf[:], obuf[:], g2[:])
                nc.vector.tensor_add(obuf[:], obuf[:], b2[:])

                nc.sync.dma_start(out_d.ap()[t], obuf[:])

    nc.compile()
    return nc


def _get_program():
    global _PROGRAM
    if _PROGRAM is None:
        _PROGRAM = _build_program()
    return _PROGRAM


def _prepare_inputs(x, ln1_g, ln1_b, ln2_g, ln2_b, node_idx, stacked_indices):
    """Host-side index preprocessing + weight prep. Returns list of in_maps."""
    node_idx = np.asarray(node_idx).astype(np.int64)
    stacked = np.asarray(stacked_indices).astype(np.int64)
    x = np.asarray(x, dtype=np.float32)
    ln1_g = np.asarray(ln1_g, dtype=np.float32)
    ln1_b = np.asarray(ln1_b, dtype=np.float32)
    ln2_g = np.asarray(ln2_g, dtype=np.float32)
    ln2_b = np.asarray(ln2_b, dtype=np.float32)

    # histogram bt[n, j] = count of node n in basket j  (index preprocessing)
    bt = np.zeros((NUM_NODES, COMP_LEN), dtype=np.float32)
    j_ids = np.broadcast_to(np.arange(COMP_LEN)[:, None], stacked.shape)
    np.add.at(bt, (stacked.ravel(), j_ids.ravel()), 1.0)

    # weight prep
    wstat = np.zeros((E, NSTAT), dtype=np.float32)
    wstat[np.arange(E), np.arange(E) // 4] = ln1_g
    wstat[:, 32] = 1.0
    wstat_bf = wstat.astype(BF16)
    scv = ln1_g.reshape(COMP_DIM, 4).sum(1)
    bbv = ln1_b.reshape(COMP_DIM, 4).mean(1)
    sc782 = np.broadcast_to(scv / MAX_LEN, (COMP_LEN, COMP_DIM)).copy()
    bb782 = np.broadcast_to(bbv / MAX_LEN, (COMP_LEN, COMP_DIM)).copy()
    g2 = np.ascontiguousarray(ln2_g.reshape(COMP_LEN, COMP_DIM))
    b2 = np.ascontiguousarray(ln2_b.reshape(COMP_LEN, COMP_DIM))

    in_maps = []
    for core in range(N_CORES):
        ts = list(range(core * T_LOC, (core + 1) * T_LOC))
        # A[i, j] per timestamp, laid out [128 p, CH g, 64 j] (token = g*128+p)
        am = np.empty((T_LOC, 128, CH, COMP_LEN), dtype=BF16)
        for ti, tg in enumerate(ts):
            a_full = bt[node_idx[tg, :N_NODE], :]          # [8192, 64]
            am[ti] = a_full.reshape(CH, 128, COMP_LEN).transpose(1, 0, 2)
        xb = x[ts, :N_NODE, :].astype(BF16)
        in_maps.append({
            "xb": xb,
            "am": am,
            "wstat": wstat_bf,
            "sc782": sc782.astype(np.float32),
            "bb782": bb782.astype(np.float32),
            "g2": g2.astype(np.float32),
            "b2": b2.astype(np.float32),
        })
    return in_maps


def kernel(x, ln1_g, ln1_b, ln2_g, ln2_b, node_idx, stacked_indices,
           n_node=N_NODE, num_nodes=NUM_NODES):
    global LAST_RESULTS
    from concourse.bass_utils import run_bass_kernel_spmd

    nc = _get_program()
    in_maps = _prepare_inputs(x, ln1_g, ln1_b, ln2_g, ln2_b, node_idx,
                              stacked_indices)

    if os.environ.get("KERNEL_SIM"):
        outs = _run_sim(nc, in_maps)
    else:
        res = run_bass_kernel_spmd(
            nc, in_maps, core_ids=list(range(N_CORES)),
            trace=bool(os.environ.get("KERNEL_TRACE")),
        )
        LAST_RESULTS = res
        outs = [r["out"] for r in res.results]

    full = np.concatenate(outs, axis=0)           # [16, 64, 32]
    return full.reshape(T, 1, COMP_LEN * COMP_DIM).astype(np.float32)


def _run_sim(nc, in_maps):
    """CoreSim path (KERNEL_SIM=1): simulate cores serially."""
    from concourse.bass_interp import CoreSim
    outs = []
    ncores = int(os.environ.get("KERNEL_SIM_CORES", "1"))
    for core, im in enumerate(in_maps[:ncores]):
        sim = CoreSim(nc, trace=False)
        for k, v in im.items():
            sim.tensor(k)[:] = v
        sim.simulate(check_with_hw=False)
        outs.append(np.array(sim.tensor("out")))
    for core in range(ncores, len(in_maps)):
        outs.append(np.zeros((T_LOC, COMP_LEN, COMP_DIM), np.float32))
    return outs


# revision 15
# speedup vs baseline: 3.2874x; 1.2730x over previous
"""Trainium2 Bass kernel for nn_Disentangler (gnn_message_passing).

Reference computation per timestamp t (T=16):
  xn   = LayerNorm_E(x[t])                 [16384, 128] -> first 8192 rows used
  tee  = segment_sum(xn[:8192] by node_idx[t])      [50000, 128]
  pool = blockmean_4(tee)                           [50000, 32]
  agg  = mean over basket slots of pool[stacked]    [64, 32]
  out  = LayerNorm_2048(agg.reshape(1, 2048))

Algebraic reformulation (all FP math on x happens on device):
  For token i with node n_i, A[i, j] = (# occurrences of n_i among basket j's
  782 slots) — an integer count matrix derived purely from the two index
  tensors (host-side index preprocessing).  With per-token LN1 stats
  (m_i, r_i = rsqrt(var_i+eps)), q_i[c] = sum_{e in block c} x[i,e]*g1[e],
  sc[c] = sum_block g1, bb[c] = mean_block b1:

    agg[j, c] = (1/782) * [ sum_i A[i,j]*u_i[c]        (u = q * r/4)
                            - sc[c] * sum_i A[i,j]*w_i  (w = m * r/4)
                            + bb[c] * sum_i A[i,j] ]

  i.e. one token-contraction matmul  A^T @ [u | 1 | w]  per timestamp.
  Tokens whose node appears in no basket have A == 0 and are dropped
  host-side (packed token list, ~5.2k of 8192; padded to NT=5632).

Sharding: data-parallel over T (2 timestamps per core, 8 cores).

Device pipeline per timestamp:
  1. xT [E=128, NT] bf16 <- HWDGE dma_start_transpose of packed x rows,
     in 4 pieces so stats matmuls pipeline with the load
  2. sq chunks = xT*xT (DVE, per 512-token chunk)
  3. stats: per 512-chunk, [0|ssq] selector matmul lands on PSUM rows 32-33
     (base 32), then the 33-col [Wg|1] matmul at base 0 overwrites row 32
     with sum_x (program-order WAW) -> one [34, 512] PSUM tile per chunk,
     evacuated alternately by ACT/DVE to stats_e [34, NT] bf16
  4. 44 PE transposes [34,128]->[128,34] -> token-major stats
  5. tiny token-major DVE/ACT ops -> r4, u, w  (rhs2 = [u | 1 | w] bf16)
  6. 44 accumulating matmuls psC[64,34] = A-chunk^T @ rhs2-chunk
  7. agg finalize + LayerNorm(2048); global sums + broadcast via three tiny
     matmuls; output [64, 32] f32 -> HBM.
"""

import os
import sys

import ml_dtypes
import numpy as np

# ---------------------------------------------------------------- constants
T = 16
TOK = 16384
E = 128
N_NODE = 8192
NUM_NODES = 50000
COMP_LEN = 64   # J baskets
MAX_LEN = 782
COMP_DIM = 32   # C
EPS = 1e-5

N_CORES = 8
T_LOC = T // N_CORES   # 2 timestamps per core

NT = 5632              # packed tokens (kept ~5186 +- 44; 10 sigma headroom)
CH = NT // 128         # 44 token chunks
NK = NT // 512         # 11 stats matmul chunks
NSTAT = 34             # stats rows: [q(32) | sum_x | sum_x2]
BTR = 8                # back-transposes batched per PSUM tile
XPIECES = (1536, 1536, 1536, 1024)   # xT load pieces (multiples of 512)

_PROGRAM = None
LAST_RESULTS = None    # BassKernelResults of the last run (for test harness)

BF16 = ml_dtypes.bfloat16


def _build_program():
    import concourse.bacc as bacc
    import concourse.bass as bass
    import concourse.mybir as mybir
    import concourse.tile as tile
    from concourse import masks

    f32 = mybir.dt.float32
    bf16 = mybir.dt.bfloat16

    nc = bacc.Bacc("TRN2", target_bir_lowering=False, debug=False,
                   num_devices=N_CORES)

    xb_d = nc.dram_tensor("xb", [T_LOC, NT, E], bf16, kind="ExternalInput")
    am_d = nc.dram_tensor("am", [T_LOC, 128, CH, COMP_LEN], bf16,
                          kind="ExternalInput")
    wstat_d = nc.dram_tensor("wstat", [E, NSTAT], bf16, kind="ExternalInput")
    sc_d = nc.dram_tensor("sc782", [COMP_LEN, COMP_DIM], f32,
                          kind="ExternalInput")
    bb_d = nc.dram_tensor("bb782", [COMP_LEN, COMP_DIM], f32,
                          kind="ExternalInput")
    g2_d = nc.dram_tensor("g2", [COMP_LEN, COMP_DIM], f32, kind="ExternalInput")
    b2_d = nc.dram_tensor("b2", [COMP_LEN, COMP_DIM], f32, kind="ExternalInput")
    out_d = nc.dram_tensor("out", [T_LOC, COMP_LEN, COMP_DIM], f32,
                           kind="ExternalOutput")

    with tile.TileContext(nc) as tc:
        with (
            tc.tile_pool(name="const", bufs=1) as cp,
            tc.tile_pool(name="main", bufs=2) as pool,
            tc.tile_pool(name="small", bufs=2) as sp,
            tc.tile_pool(name="ps", bufs=3, space=bass.MemorySpace.PSUM) as psp,
            tc.tile_pool(name="pst", bufs=2, space=bass.MemorySpace.PSUM) as pstp,
            tc.tile_pool(name="psc", bufs=2, space=bass.MemorySpace.PSUM) as pscp,
            tc.tile_pool(name="psde", bufs=1, space=bass.MemorySpace.PSUM) as psdep,
        ):
            # ---- constants
            wstat = cp.tile([E, NSTAT], bf16)
            nc.sync.dma_start(wstat[:], wstat_d.ap())
            sc = cp.tile([COMP_LEN, COMP_DIM], f32)
            nc.sync.dma_start(sc[:], sc_d.ap())
            bb = cp.tile([COMP_LEN, COMP_DIM], f32)
            nc.sync.dma_start(bb[:], bb_d.ap())
            g2 = cp.tile([COMP_LEN, COMP_DIM], f32)
            nc.sync.dma_start(g2[:], g2_d.ap())
            b2 = cp.tile([COMP_LEN, COMP_DIM], f32)
            nc.sync.dma_start(b2[:], b2_d.ap())
            ones64 = cp.tile([COMP_LEN, 1], f32)
            nc.gpsimd.memset(ones64[:], 1.0)
            onesrow = cp.tile([1, COMP_LEN], f32)
            nc.gpsimd.memset(onesrow[:], 1.0)
            sel2 = cp.tile([COMP_LEN, 2], f32)
            nc.gpsimd.memset(sel2[:], 0.0)
            nc.gpsimd.memset(sel2[0:COMP_DIM, 0:1], 1.0)
            nc.gpsimd.memset(sel2[COMP_DIM:COMP_LEN, 1:2], 1.0)
            epsb = cp.tile([128, 1], f32)
            nc.gpsimd.memset(epsb[:], EPS)
            ident = cp.tile([128, 128], bf16)
            masks.make_identity(nc, ident[:])
            # [zero | ones] selector: lhsT for the sum_x2 row
            ssqsel = cp.tile([E, 2], bf16)
            nc.gpsimd.memset(ssqsel[:, 0:1], 0.0)
            nc.gpsimd.memset(ssqsel[:, 1:2], 1.0)
            warm = cp.tile([E, 512], bf16)
            nc.gpsimd.memset(warm[:], 0.5)

            # PE p-state warmup burst (~4 us) while the first x piece loads
            psw = psp.tile([NSTAT, 512], f32, tag="psA")
            for _ in range(18):
                nc.tensor.matmul(psw[0:33, :], wstat[:, 0:33], warm[:],
                                 start=True, stop=True)

            for t in range(T_LOC):
                # ---- 1. transposed load of packed x rows, in pieces
                xT = pool.tile([128, NT], bf16, tag="xT")
                off = 0
                for plen in XPIECES:
                    nc.sync.dma_start_transpose(
                        xT[:, off:off + plen],
                        xb_d.ap()[t, off:off + plen, :])
                    off += plen

                # ---- A matrix (host-prepared counts, chunk layout)
                a_sb = pool.tile([128, CH, COMP_LEN], bf16, tag="A")
                nc.sync.dma_start(a_sb[:], am_d.ap()[t])

                # ---- 2+3. per-chunk square + stats matmuls
                sqT = pool.tile([128, NT], bf16, tag="sqT")
                stats_e = pool.tile([NSTAT, NT], bf16, tag="stats_e")
                for k in range(NK):
                    ksl = slice(k * 512, (k + 1) * 512)
                    nc.vector.tensor_mul(sqT[:, ksl], xT[:, ksl], xT[:, ksl])
                    ps = psp.tile([NSTAT, 512], f32, tag="psA")
                    nc.tensor.matmul(ps[32:34, :], ssqsel[:], sqT[:, ksl],
                                     start=True, stop=True)
                    nc.tensor.matmul(ps[0:33, :], wstat[:, 0:33], xT[:, ksl],
                                     start=True, stop=True)
                    if k % 2 == 0:
                        nc.scalar.copy(stats_e[:, ksl], ps[:])
                    else:
                        nc.vector.tensor_copy(stats_e[:, ksl], ps[:])

                # ---- 4. back-transpose stats to token-major
                stats_tok = pool.tile([128, CH, NSTAT], bf16, tag="stats_tok")
                for g0 in range(0, CH, BTR):
                    nb = min(BTR, CH - g0)
                    pstk = pstp.tile([128, BTR, NSTAT], bf16, tag="psTk")
                    for j in range(nb):
                        g = g0 + j
                        nc.tensor.transpose(
                            pstk[:, j, :],
                            stats_e[:, g * 128:(g + 1) * 128],
                            ident[0:NSTAT, 0:NSTAT])
                    nc.vector.tensor_copy(stats_tok[:, g0:g0 + nb, :],
                                          pstk[:, 0:nb, :])

                # ---- 5. per-token scalars (all [128, CH], tiny)
                m_f = sp.tile([128, CH], f32, tag="m")
                nc.vector.tensor_scalar_mul(m_f[:], stats_tok[:, :, 32], 1.0 / E)
                v_f = sp.tile([128, CH], f32, tag="v")
                nc.vector.tensor_scalar_mul(v_f[:], stats_tok[:, :, 33], 1.0 / E)
                m2_f = sp.tile([128, CH], f32, tag="m2")
                nc.vector.tensor_mul(m2_f[:], m_f[:], m_f[:])
                nc.vector.tensor_sub(v_f[:], v_f[:], m2_f[:])
                sd_f = sp.tile([128, CH], f32, tag="sd")
                nc.scalar.activation(sd_f[:], v_f[:],
                                     mybir.ActivationFunctionType.Sqrt,
                                     bias=epsb[:])
                ri_f = sp.tile([128, CH], f32, tag="ri")
                nc.vector.reciprocal(ri_f[:], sd_f[:])
                r4_b = sp.tile([128, CH], bf16, tag="r4")
                nc.vector.tensor_scalar_mul(r4_b[:], ri_f[:], 0.25)
                w_f = sp.tile([128, CH], f32, tag="w")
                nc.vector.tensor_mul(w_f[:], m_f[:], ri_f[:])

                rhs2 = pool.tile([128, CH, NSTAT], bf16, tag="rhs2")
                nc.vector.tensor_mul(
                    rhs2[:, :, 0:COMP_DIM], stats_tok[:, :, 0:COMP_DIM],
                    r4_b[:].unsqueeze(2).broadcast_to([128, CH, COMP_DIM]))
                nc.gpsimd.memset(rhs2[:, :, 32:33], 1.0)
                nc.vector.tensor_scalar_mul(rhs2[:, :, 33], w_f[:], 0.25)

                # ---- 6. token contraction
                psc = pscp.tile([COMP_LEN, NSTAT], f32, tag="psC")
                for g in range(CH):
                    nc.tensor.matmul(psc[:], a_sb[:, g, :], rhs2[:, g, :],
                                     start=(g == 0), stop=(g == CH - 1))

                # ---- 7. agg finalize ([64, 32] fp32, tiny)
                cat = sp.tile([COMP_LEN, NSTAT], f32, tag="cat")
                nc.scalar.copy(cat[:], psc[:])
                t1 = sp.tile([COMP_LEN, COMP_DIM], f32, tag="t1")
                nc.vector.tensor_mul(
                    t1[:], cat[:, 33:34].broadcast_to([COMP_LEN, COMP_DIM]),
                    sc[:])
                t2 = sp.tile([COMP_LEN, COMP_DIM], f32, tag="t2")
                nc.vector.tensor_mul(
                    t2[:], cat[:, 32:33].broadcast_to([COMP_LEN, COMP_DIM]),
                    bb[:])
                nc.vector.tensor_sub(t2[:], t2[:], t1[:])
                t0 = sp.tile([COMP_LEN, COMP_DIM], f32, tag="t0")
                nc.vector.tensor_scalar_mul(t0[:], cat[:, 0:COMP_DIM],
                                            1.0 / MAX_LEN)
                cat2 = sp.tile([COMP_LEN, 2 * COMP_DIM], f32, tag="cat2")
                nc.vector.tensor_add(cat2[:, 0:COMP_DIM], t0[:], t2[:])
                nc.vector.tensor_mul(cat2[:, COMP_DIM:2 * COMP_DIM],
                                     cat2[:, 0:COMP_DIM], cat2[:, 0:COMP_DIM])

                # ---- LN2 global sums + broadcast via PE
                psd = psdep.tile([COMP_LEN, 1], f32, tag="psDE")
                nc.tensor.matmul(psd[:], cat2[:], ones64[:],
                                 start=True, stop=True)
                sD = sp.tile([COMP_LEN, 1], f32, tag="sD")
                nc.vector.tensor_copy(sD[:], psd[:])
                pse = psdep.tile([1, 2], f32, tag="psDE")
                nc.tensor.matmul(pse[:], sD[:], sel2[:], start=True, stop=True)
                sE = sp.tile([1, 2], f32, tag="sE")
                nc.vector.tensor_copy(sE[:], pse[:])
                # broadcast [1, 2] -> [64, 2] with a K=1 matmul
                psf = psdep.tile([COMP_LEN, 2], f32, tag="psDE")
                nc.tensor.matmul(psf[:], onesrow[:], sE[:],
                                 start=True, stop=True)
                bS = sp.tile([COMP_LEN, 2], f32, tag="bS")
                nc.vector.tensor_copy(bS[:], psf[:])

                NTOT = float(COMP_LEN * COMP_DIM)
                mu = sp.tile([COMP_LEN, 1], f32, tag="mu")
                nc.vector.tensor_scalar_mul(mu[:], bS[:, 0:1], 1.0 / NTOT)
                ex2 = sp.tile([COMP_LEN, 1], f32, tag="ex2")
                nc.vector.tensor_scalar_mul(ex2[:], bS[:, 1:2], 1.0 / NTOT)
                mu2 = sp.tile([COMP_LEN, 1], f32, tag="mu2")
                nc.vector.tensor_mul(mu2[:], mu[:], mu[:])
                nc.vector.tensor_sub(ex2[:], ex2[:], mu2[:])
                sd2 = sp.tile([COMP_LEN, 1], f32, tag="sd2")
                nc.scalar.activation(sd2[:], ex2[:],
                                     mybir.ActivationFunctionType.Sqrt,
                                     bias=epsb[0:COMP_LEN, :])
                rr = sp.tile([COMP_LEN, 1], f32, tag="rr")
                nc.vector.reciprocal(rr[:], sd2[:])

                obuf = sp.tile([COMP_LEN, COMP_DIM], f32, tag="obuf")
                nc.vector.tensor_scalar(obuf[:], cat2[:, 0:COMP_DIM],
                                        mu[:], rr[:],
                                        mybir.AluOpType.subtract,
                                        mybir.AluOpType.mult)
                nc.vector.tensor_mul(obuf[:], obuf[:], g2[:])
                nc.vector.tensor_add(obuf[:], obuf[:], b2[:])

                nc.sync.dma_start(out_d.ap()[t], obuf[:])

    nc.compile()
    return nc


def _get_program():
    global _PROGRAM
    if _PROGRAM is None:
        _PROGRAM = _build_program()
    return _PROGRAM


def _prepare_inputs(x, ln1_g, ln1_b, ln2_g, ln2_b, node_idx, stacked_indices):
    """Host-side index preprocessing + weight prep. Returns list of in_maps."""
    node_idx = np.asarray(node_idx).astype(np.int64)
    stacked = np.asarray(stacked_indices).astype(np.int64)
    x = np.asarray(x, dtype=np.float32)
    ln1_g = np.asarray(ln1_g, dtype=np.float32)
    ln1_b = np.asarray(ln1_b, dtype=np.float32)
    ln2_g = np.asarray(ln2_g, dtype=np.float32)
    ln2_b = np.asarray(ln2_b, dtype=np.float32)

    # histogram bt[n, j] = count of node n in basket j  (index preprocessing)
    bt = np.zeros((NUM_NODES, COMP_LEN), dtype=np.float32)
    j_ids = np.broadcast_to(np.arange(COMP_LEN)[:, None], stacked.shape)
    np.add.at(bt, (stacked.ravel(), j_ids.ravel()), 1.0)
    node_used = bt.any(axis=1)

    # weight prep
    wstat = np.zeros((E, NSTAT), dtype=np.float32)
    wstat[np.arange(E), np.arange(E) // 4] = ln1_g
    wstat[:, 32] = 1.0
    wstat_bf = wstat.astype(BF16)
    scv = ln1_g.reshape(COMP_DIM, 4).sum(1)
    bbv = ln1_b.reshape(COMP_DIM, 4).mean(1)
    sc782 = np.broadcast_to(scv / MAX_LEN, (COMP_LEN, COMP_DIM)).copy()
    bb782 = np.broadcast_to(bbv / MAX_LEN, (COMP_LEN, COMP_DIM)).copy()
    g2 = np.ascontiguousarray(ln2_g.reshape(COMP_LEN, COMP_DIM))
    b2 = np.ascontiguousarray(ln2_b.reshape(COMP_LEN, COMP_DIM))

    in_maps = []
    for core in range(N_CORES):
        ts = list(range(core * T_LOC, (core + 1) * T_LOC))
        am = np.zeros((T_LOC, 128, CH, COMP_LEN), dtype=BF16)
        xb = np.empty((T_LOC, NT, E), dtype=BF16)
        for ti, tg in enumerate(ts):
            nt_ids = node_idx[tg, :N_NODE]
            kept = np.flatnonzero(node_used[nt_ids])
            if len(kept) > NT:
                print(f"WARNING: kept token overflow {len(kept)} > {NT}",
                      file=sys.stderr)
                kept = kept[:NT]
            nk = len(kept)
            sel = np.zeros(NT, dtype=np.int64)
            sel[:nk] = kept
            xb[ti] = x[tg, sel, :].astype(BF16)
            a_full = bt[nt_ids[sel], :]
            a_full[nk:, :] = 0.0
            am[ti] = a_full.reshape(CH, 128, COMP_LEN).transpose(1, 0, 2)
        in_maps.append({
            "xb": xb,
            "am": am,
            "wstat": wstat_bf,
            "sc782": sc782.astype(np.float32),
            "bb782": bb782.astype(np.float32),
            "g2": g2.astype(np.float32),
            "b2": b2.astype(np.float32),
        })
    return in_maps


def kernel(x, ln1_g, ln1_b, ln2_g, ln2_b, node_idx, stacked_indices,
           n_node=N_NODE, num_nodes=NUM_NODES):
    global LAST_RESULTS
    from concourse.bass_utils import run_bass_kernel_spmd

    nc = _get_program()
    in_maps = _prepare_inputs(x, ln1_g, ln1_b, ln2_g, ln2_b, node_idx,
                              stacked_indices)

    if os.environ.get("KERNEL_SIM"):
        outs = _run_sim(nc, in_maps)
    else:
        res = run_bass_kernel_spmd(
            nc, in_maps, core_ids=list(range(N_CORES)),
            trace=bool(os.environ.get("KERNEL_TRACE")),
        )
        LAST_RESULTS = res
        outs = [r["out"] for r in res.results]

    full = np.concatenate(outs, axis=0)           # [16, 64, 32]
    return full.reshape(T, 1, COMP_LEN * COMP_DIM).astype(np.float32)


def _run_sim(nc, in_maps):
    """CoreSim path (KERNEL_SIM=1): simulate cores serially."""
    from concourse.bass_interp import CoreSim
    outs = []
    ncores = int(os.environ.get("KERNEL_SIM_CORES", "1"))
    for core, im in enumerate(in_maps[:ncores]):
        sim = CoreSim(nc, trace=False)
        for k, v in im.items():
            sim.tensor(k)[:] = v
        sim.simulate(check_with_hw=False)
        outs.append(np.array(sim.tensor("out")))
    for core in range(ncores, len(in_maps)):
        outs.append(np.zeros((T_LOC, COMP_LEN, COMP_DIM), np.float32))
    return outs
